# revision 21
# baseline (speedup 1.0000x reference)
"""Trainium2 Bass kernel for nn_AdaptiveSTSamplingMixing.

8 cores: core c -> group g = c % 4, batch-pair bp = c // 4.
Group partials AllReduced over {0..3} / {4..7}.
"""
import sys
sys.path.insert(0, '/opt/trn_rl_repo')
import numpy as np
import ml_dtypes

import concourse.bass as bass
import concourse.bacc as bacc
import concourse.tile as tile
from concourse import mybir
from concourse import bass_utils

F32 = mybir.dt.float32
BF16 = mybir.dt.bfloat16
I16 = mybir.dt.int16
AL = mybir.AluOpType
AF = mybir.ActivationFunctionType
AX = mybir.AxisListType

STRIDES = (4, 8, 16, 32)
HWL = (64, 32, 16, 8)
LOFF = (0, 4096, 5120, 5376)
NPIXP = 5504
NCH = 43
SP, TP, D, CG = 32, 8, 256, 64
OUT_S, OUT_T = 128, 32
Q, T = 100, 8
EFF = 64
PS = EFF * EFF + SP * OUT_S
PT = EFF * EFF + TP * OUT_T
N_CORES = 8
NSLOT = 256
CO2ROWS = (1 + NPIXP) * 4


def _ap(t_ap, off, pattern):
    a = t_ap if isinstance(t_ap, bass.AP) else t_ap[:]
    pattern = [list(p) for p in pattern]
    try:
        space = a.tensor.space
    except Exception:
        space = None
    if space is not None and str(space) != 'MemorySpace.DRAM' and a.ap:
        # on-chip: partition pair step must be the natural per-partition pitch
        pattern[0][0] = a.ap[0][0]
    return bass.AP(tensor=a.tensor, offset=a.offset + off, ap=pattern)


def _stats(nc, pool, x, n, npart, tag):
    """returns (alpha, beta) columns s.t. normalized = x*alpha + beta"""
    nsub = (n + 511) // 512
    chunk = n // nsub
    stats = pool.tile([npart, nsub, 6], F32, tag=f"st{tag}")
    for i in range(nsub):
        nc.vector.bn_stats(out=stats[:, i, :], in_=x[:, i * chunk:(i + 1) * chunk])
    mv = pool.tile([npart, 2], F32, tag=f"mv{tag}")
    nc.vector.bn_aggr(out=mv[:], in_=stats[:])
    eps = pool.tile([npart, 1], F32, tag=f"ep{tag}")
    nc.vector.memset(eps[:], 1e-5)
    sd = pool.tile([npart, 1], F32, tag=f"sd{tag}")
    nc.scalar.activation(out=sd[:], in_=mv[:, 1:2], func=AF.Sqrt, bias=eps[:, 0:1])
    nc.vector.reciprocal(out=sd[:], in_=sd[:])
    be = pool.tile([npart, 1], F32, tag=f"be{tag}")
    nc.vector.tensor_scalar(be[:], mv[:, 0:1], sd[:, 0:1], -1.0, AL.mult, AL.mult)
    return sd, be


def _pw_ln_relu(nc, sb2, sbMp, psA, x, POUT, onesC, onesR, tag):
    """LayerNorm(+relu) over (partition, EFF-free-inner) per q, partition-wise.

    x: [*, Q, EFF] tile with POUT active partitions. In-place."""
    xsq = sbMp.tile([128, Q, EFF], BF16, tag="med")
    nc.scalar.activation(out=xsq[0:POUT, :, :], in_=x[0:POUT, :, :], func=AF.Square)
    mrow = sb2.tile([1, 2, Q], F32, tag="mrow")
    for which, srcT in ((0, x), (1, xsq)):
        for cc in range((Q * EFF + 511) // 512):
            n = min(512, Q * EFF - cc * 512)
            cp = psA.tile([1, 512], F32, tag="psa")
            nc.tensor.matmul(
                cp[0:1, 0:n], onesC[0:POUT, 0:1],
                _ap(srcT, cc * 512, [[1, POUT], [1, n]]),
                start=True, stop=True)
            nc.vector.tensor_reduce(
                out=mrow[0:1, which, cc * 8:cc * 8 + n // 64],
                in_=_ap(cp, 0, [[1, 1], [EFF, n // 64], [1, EFF]]),
                axis=AX.X, op=AL.add)
    NN = float(POUT * EFF)
    arow = sb2.tile([1, 2, Q], F32, tag="arow")
    nc.vector.tensor_scalar(mrow[0:1, 0, :], mrow[0:1, 0, :], 1.0 / NN, None, AL.mult)
    nc.vector.tensor_scalar(mrow[0:1, 1, :], mrow[0:1, 1, :], 1.0 / NN, None, AL.mult)
    mu2r = sb2.tile([1, Q], F32, tag="mu2r")
    nc.vector.tensor_tensor(out=mu2r[:], in0=mrow[0:1, 0, :], in1=mrow[0:1, 0, :],
                            op=AL.mult)
    nc.vector.tensor_tensor(out=mu2r[:], in0=mrow[0:1, 1, :], in1=mu2r[:],
                            op=AL.subtract)
    epsr = sb2.tile([1, Q], F32, tag="epsr")
    nc.vector.memset(epsr[:], 1e-5)
    nc.vector.tensor_tensor(out=mu2r[:], in0=mu2r[:], in1=epsr[:], op=AL.add)
    nc.scalar.activation(out=mu2r[:], in_=mu2r[:], func=AF.Sqrt)
    nc.vector.reciprocal(out=mu2r[:], in_=mu2r[:])
    nc.vector.tensor_copy(out=arow[0:1, 0, :], in_=mu2r[:])
    nc.vector.tensor_tensor(out=arow[0:1, 1, :], in0=mrow[0:1, 0, :], in1=mu2r[:],
                            op=AL.mult)
    nc.vector.tensor_scalar(arow[0:1, 1, :], arow[0:1, 1, :], -1.0, None, AL.mult)
    alb = psA.tile([128, Q], F32, tag="psa")
    nc.tensor.matmul(alb[:], onesR[0:1, :], arow[0:1, 0, :], start=True, stop=True)
    beb = psA.tile([128, Q], F32, tag="psa")
    nc.tensor.matmul(beb[:], onesR[0:1, :], arow[0:1, 1, :], start=True, stop=True)
    nc.vector.tensor_tensor(
        out=x[0:POUT, :, :], in0=x[0:POUT, :, :],
        in1=_ap(alb, 0, [[1, POUT], [1, Q], [0, EFF]]), op=AL.mult)
    nc.vector.tensor_tensor(
        out=x[0:POUT, :, :], in0=x[0:POUT, :, :],
        in1=_ap(beb, 0, [[1, POUT], [1, Q], [0, EFF]]), op=AL.add)
    nc.scalar.activation(out=x[0:POUT, :, :], in_=x[0:POUT, :, :], func=AF.Relu)


def build_nc(dbg=False):
    nc = bacc.Bacc("TRN2", target_bir_lowering=False, debug=False,
                   num_devices=N_CORES, enable_asserts=False)
    dt = lambda n, s, d=F32: nc.dram_tensor(n, s, d, kind="ExternalInput").ap()
    slabs = dt("slabs", [2, NPIXP, T * CG], BF16)
    sqin = dt("sq", [2, Q, D]); tqin = dt("tq", [2, Q, D])
    boxin = dt("boxes", [2, Q, 4])
    w_off = dt("w_off", [D, 96]); boffbc = dt("boffbc", [128, 96])
    w_ps = dt("w_ps", [D, PS]); w_pt = dt("w_pt", [D, PT])
    wod_s = dt("wod_s", [EFF * OUT_S, D]); wod_t = dt("wod_t", [EFF * OUT_T, D])
    mb_s = dt("mb_s", [EFF, EFF], BF16); sbt_s = dt("sbt_s", [SP, OUT_S], BF16)
    mb_t = dt("mb_t", [EFF, EFF], BF16); sbt_t = dt("sbt_t", [TP, OUT_T], BF16)
    ident = dt("ident", [128, 128])
    ohbase = dt("ohbase", [128, 64])
    shcol = dt("shcol", [128, 1])
    sel0 = dt("sel0", [Q, 128]); sel1 = dt("sel1", [Q, 128])
    onesin = dt("onesin", [128, 1])
    gsbc = dt("gsbc", [128, D]); besbc = dt("besbc", [128, D])
    gtbc = dt("gtbc", [128, D]); betbc = dt("betbc", [128, D])
    bosbc = dt("bosbc", [128, D]); botbc = dt("botbc", [128, D])
    out = nc.dram_tensor("out", [2, 2, Q, D], F32, kind="ExternalOutput").ap()
    if dbg:
        d_spat = nc.dram_tensor("dbg_spat", [2, Q, SP * CG], F32,
                                kind="ExternalOutput").ap()
        d_temp = nc.dram_tensor("dbg_temp", [2, Q, T * CG], F32,
                                kind="ExternalOutput").ap()

    di = lambda n, s, d=F32: nc.dram_tensor(n, s, d, kind="Internal").ap()
    mslab_d = di("mslab_d", [2, NPIXP + 8, CG])
    co2_d = di("co2_d", [2, CO2ROWS, 64])
    idx_d = di("idx_d", [2, 3, 128, NSLOT], I16)
    spatT_d = di("spatT_d", [2, CG, Q, SP])
    tempT_d = di("tempT_d", [2, CG, Q, TP])
    sT_d = di("sT_d", [SP, Q, OUT_S], BF16)
    par_d = di("par_d", [2, 2, Q, PS], BF16)
    ar_in = di("ar_in", [2, 2, Q, D])
    ar_out = di("ar_out", [2, 2, Q, D])

    with tile.TileContext(nc) as tc:
        import contextlib
        ctx = contextlib.ExitStack()
        with ctx:
            sb1 = ctx.enter_context(tc.tile_pool(name="sb1", bufs=1))
            sbS = ctx.enter_context(tc.tile_pool(name="sbS", bufs=2))
            sbM = ctx.enter_context(tc.tile_pool(name="sbM", bufs=2))
            sb2 = ctx.enter_context(tc.tile_pool(name="sb2", bufs=1))
            sb3 = ctx.enter_context(tc.tile_pool(name="sb3", bufs=2))
            sb4 = ctx.enter_context(tc.tile_pool(name="sb4", bufs=1))
            psA = ctx.enter_context(tc.tile_pool(name="psA", bufs=2, space="PSUM"))
            psB = ctx.enter_context(tc.tile_pool(name="psB", bufs=2, space="PSUM"))

            def cload(src, shape, dtyp=F32, tag=None):
                t = sb1.tile(shape, dtyp, tag=tag)
                nc.sync.dma_start(out=t[:], in_=src)
                return t
            identS = cload(ident, [128, 128], tag="identS")
            ohbS = cload(ohbase, [128, 64], tag="ohbS")
            shcS = cload(shcol, [128, 1], tag="shcS")
            selS = [cload(sel0, [Q, 128], tag="sel0S"),
                    cload(sel1, [Q, 128], tag="sel1S")]
            boffS = cload(boffbc, [128, 96], tag="boffS")
            gbcS = [cload(gsbc, [128, D], tag="gs"), cload(gtbc, [128, D], tag="gt")]
            bebcS = [cload(besbc, [128, D], tag="bes"), cload(betbc, [128, D], tag="bet")]
            bobcS = [cload(bosbc, [128, D], tag="bos"), cload(botbc, [128, D], tag="bot")]
            onesC = sb1.tile([128, 1], BF16, tag="onesC")
            onesCf = cload(onesin, [128, 1], tag="onesCf")
            nc.vector.tensor_copy(out=onesC[:], in_=onesCf[:])
            onesR = sb1.tile([1, 128], F32, tag="onesR")
            nc.vector.memset(onesR[:], 1.0)
            woffS = sb1.tile([128, 2, 96], F32, tag="woff")
            nc.sync.dma_start(out=woffS[:], in_=_ap(w_off, 0,
                              [[96, 128], [96 * 128, 2], [1, 96]]))
            mbSb = [cload(mb_s, [EFF, EFF], BF16, tag="mb0"),
                    cload(mb_t, [EFF, EFF], BF16, tag="mb1")]
            sbtSb = [cload(sbt_s, [SP, OUT_S], BF16, tag="sbt0"),
                     cload(sbt_t, [TP, OUT_T], BF16, tag="sbt1")]

            zt = sb1.tile([128, 256], F32, tag="zt")
            nc.vector.memset(zt[:], 0.0)

            qTs = {}
            qS = {}
            for b in range(2):
                tot = CO2ROWS * 64
                CH0 = 128 * 256
                for i in range((tot + CH0 - 1) // CH0):
                    n = min(CH0, tot - i * CH0)
                    rows = n // 256
                    if rows:
                        nc.sync.dma_start(
                            out=_ap(co2_d, b * tot + i * CH0,
                                    [[256, rows], [1, 256]]),
                            in_=zt[0:rows, :])
                    if n % 256:
                        nc.sync.dma_start(
                            out=_ap(co2_d, b * tot + i * CH0 + rows * 256,
                                    [[1, n % 256]]),
                            in_=zt[0:1, 0:(n % 256)])
                for ty, qin in ((0, sqin), (1, tqin)):
                    qt = sb2.tile([Q, D], F32, tag="qtt")
                    nc.sync.dma_start(out=qt[:], in_=_ap(qin, b * Q * D, [[D, Q], [1, D]]))
                    qTt = sb1.tile([128, 2, Q], F32, tag=f"qT{ty}{b}")
                    qTtb = sb1.tile([128, 2, Q], BF16, tag=f"qTb{ty}{b}")
                    for ch in range(2):
                        pt = psA.tile([128, Q], F32, tag="psa")
                        nc.tensor.transpose(pt[:], qt[:, ch * 128:(ch + 1) * 128],
                                            identS[0:Q, 0:Q])
                        nc.scalar.activation(out=qTt[:, ch, :], in_=pt[:], func=AF.Copy)
                        nc.vector.tensor_copy(out=qTtb[:, ch, :], in_=pt[:])
                    qTs[(ty, b)] = qTt
                    qTs[(ty, b, 'b')] = qTtb

            def slab_group(b, g8):
                """load slab chunks [g8*8 .. ) as [128, n, 512] bf16"""
                n = min(8, NCH - g8 * 8)
                sg = sbS.tile([128, 8, 512], BF16, tag="slabg")
                nc.sync.dma_start(
                    out=sg[0:128, 0:n, :],
                    in_=_ap(slabs, (b * NPIXP + g8 * 8 * 128) * 512,
                            [[512, 128], [512 * 128, n], [1, 512]]))
                return sg, n

            # ============ per-batch sampling ============
            for b in range(2):
                # meanT (x8): macc = sum_t slab
                macc = sbM.tile([128, NCH, CG], F32, tag="med")
                for g8 in range(6):
                    sg, n = slab_group(b, g8)
                    first = (g8 == 0)
                    for t_ in range(T):
                        if first and t_ == 0:
                            nc.vector.tensor_copy(
                                out=macc[:, g8 * 8:g8 * 8 + n, :],
                                in_=_ap(sg, t_ * CG, [[1, 128], [512, n], [1, CG]]))
                        else:
                            nc.vector.tensor_tensor(
                                out=macc[:, g8 * 8:g8 * 8 + n, :],
                                in0=macc[:, g8 * 8:g8 * 8 + n, :],
                                in1=_ap(sg, t_ * CG, [[1, 128], [512, n], [1, CG]]),
                                op=AL.add)
                nc.sync.dma_start(
                    out=_ap(mslab_d, b * (NPIXP + 8) * CG,
                            [[CG, 128], [CG * 128, NCH], [1, CG]]),
                    in_=macc[:])

                offp = psA.tile([Q, 96], F32, tag="psa")
                for ch in range(2):
                    nc.tensor.matmul(offp[:], qTs[(0, b)][:, ch, 0:Q], woffS[:, ch, :],
                                     start=(ch == 0), stop=(ch == 1))
                off = sb2.tile([Q, 96], F32, tag="off")
                nc.vector.tensor_tensor(out=off[:], in0=offp[:], in1=boffS[0:Q, :], op=AL.add)

                bx_t = sb2.tile([Q, 4], F32, tag="boxes")
                nc.sync.dma_start(out=bx_t[:], in_=_ap(boxin, b * Q * 4, [[4, Q], [1, 4]]))
                colp = sb2.tile([Q, 8], F32, tag="colp")
                LN2C = float(np.log(2.0))
                nc.scalar.activation(out=colp[:, 0:1], in_=bx_t[:, 2:3], func=AF.Exp, scale=LN2C)
                nc.scalar.activation(out=colp[:, 1:2], in_=bx_t[:, 3:4], func=AF.Exp, scale=-0.5 * LN2C)
                nc.scalar.activation(out=colp[:, 2:3], in_=bx_t[:, 3:4], func=AF.Exp, scale=0.5 * LN2C)
                nc.vector.tensor_tensor(out=colp[:, 1:2], in0=colp[:, 0:1], in1=colp[:, 1:2], op=AL.mult)
                nc.vector.tensor_tensor(out=colp[:, 2:3], in0=colp[:, 0:1], in1=colp[:, 2:3], op=AL.mult)

                lwt = sb2.tile([Q, SP, 4], F32, tag="lwt")
                mapd = sb2.tile([Q, SP], F32, tag="mapd")
                nc.vector.tensor_scalar(mapd[:], off[:, 64:96], bx_t[:, 2:3], -2.0,
                                        AL.add, AL.add)
                tmpm = sb2.tile([Q, SP], F32, tag="tmpm")
                for l in range(4):
                    nc.vector.tensor_scalar(tmpm[:], mapd[:], float(-l), None, AL.add)
                    nc.scalar.activation(out=tmpm[:], in_=tmpm[:], func=AF.Square)
                    nc.scalar.activation(out=_ap(lwt, l, [[1, Q], [4, SP]]),
                                         in_=tmpm[:], func=AF.Exp, scale=-0.5)
                lsum = sb2.tile([Q, SP], F32, tag="lsum")
                nc.vector.tensor_reduce(out=lsum[:], in_=lwt[:], axis=AX.X, op=AL.add)
                nc.vector.reciprocal(out=lsum[:], in_=lsum[:])
                # fold the 1/8 (meanT) into lw
                nc.vector.tensor_scalar(lsum[:], lsum[:], 0.125, None, AL.mult)
                for l in range(4):
                    nc.vector.tensor_tensor(
                        out=_ap(lwt, l, [[1, Q], [4, SP]]),
                        in0=_ap(lwt, l, [[1, Q], [4, SP]]), in1=lsum[:], op=AL.mult)

                IDX = sb2.tile([128, SP, 4, 2], F32, tag="IDX")
                WS = sb2.tile([128, SP, 4, 2, 2], F32, tag="WS")
                nc.vector.memset(IDX[:], 0.0)
                nc.vector.memset(WS[:], 0.0)
                for l in range(4):
                    Wl = HWL[l]
                    wtmp = {}
                    for ax, o0, whc, ccc in ((0, 0, colp[:, 1:2], bx_t[:, 0:1]),
                                             (1, 32, colp[:, 2:3], bx_t[:, 1:2])):
                        p_ = sb2.tile([Q, SP], F32, tag=f"p{ax}")
                        nc.vector.scalar_tensor_tensor(
                            out=p_[:], in0=off[:, o0:o0 + 32], scalar=whc,
                            in1=_ap(ccc, 0, [[1, Q], [0, SP]]),
                            op0=AL.mult, op1=AL.add)
                        nc.vector.tensor_scalar(p_[:], p_[:], 1.0 / STRIDES[l], 15.5,
                                                AL.mult, AL.add)
                        xi = sb2.tile([Q, SP], I16, tag=f"xi{ax}")
                        nc.vector.tensor_copy(out=xi[:], in_=p_[:])
                        xf = sb2.tile([Q, SP], F32, tag=f"xf{ax}")
                        nc.vector.tensor_copy(out=xf[:], in_=xi[:])
                        fx = sb2.tile([Q, SP], F32, tag=f"fx{ax}")
                        nc.vector.tensor_tensor(out=fx[:], in0=p_[:], in1=xf[:], op=AL.subtract)
                        bxc = sb2.tile([Q, SP], F32, tag=f"bx{ax}")
                        nc.vector.tensor_scalar(bxc[:], xf[:], 16.0, float(Wl - 2 + 16),
                                                AL.max, AL.min)
                        inr = sb2.tile([Q, SP], F32, tag=f"inr{ax}")
                        nc.vector.tensor_tensor(out=inr[:], in0=bxc[:], in1=xf[:], op=AL.is_equal)
                        eqm = sb2.tile([Q, SP], F32, tag=f"eqm{ax}")
                        nc.vector.tensor_scalar(eqm[:], xf[:], 15.0, None, AL.is_equal)
                        eqw = sb2.tile([Q, SP], F32, tag=f"eqw{ax}")
                        nc.vector.tensor_scalar(eqw[:], xf[:], float(Wl - 1 + 16), None,
                                                AL.is_equal)
                        fxm = sb2.tile([Q, SP], F32, tag=f"fxm{ax}")
                        nc.vector.tensor_scalar(fxm[:], fx[:], -1.0, 1.0, AL.mult, AL.add)
                        wA = sb2.tile([Q, SP], F32, tag=f"wA{ax}")
                        nc.vector.tensor_tensor(out=wA[:], in0=fxm[:], in1=inr[:], op=AL.mult)
                        t2 = sb2.tile([Q, SP], F32, tag=f"t2{ax}")
                        nc.vector.tensor_tensor(out=t2[:], in0=fx[:], in1=eqm[:], op=AL.mult)
                        nc.vector.tensor_tensor(out=wA[:], in0=wA[:], in1=t2[:], op=AL.add)
                        wB = sb2.tile([Q, SP], F32, tag=f"wB{ax}")
                        nc.vector.tensor_tensor(out=wB[:], in0=fx[:], in1=inr[:], op=AL.mult)
                        nc.vector.tensor_tensor(out=t2[:], in0=fxm[:], in1=eqw[:], op=AL.mult)
                        nc.vector.tensor_tensor(out=wB[:], in0=wB[:], in1=t2[:], op=AL.add)
                        wtmp[(ax, 'A')] = wA; wtmp[(ax, 'B')] = wB; wtmp[(ax, 'b')] = bxc
                    base = sb2.tile([Q, SP], F32, tag="base")
                    nc.vector.scalar_tensor_tensor(
                        out=base[:], in0=wtmp[(1, 'b')][:], scalar=float(Wl),
                        in1=wtmp[(0, 'b')][:], op0=AL.mult, op1=AL.add)
                    nc.vector.tensor_scalar(base[:], base[:],
                                            float(LOFF[l] - 17 * Wl - 16), None, AL.add)
                    for rr, wyk in ((0, 'A'), (1, 'B')):
                        wyl = sb2.tile([Q, SP], F32, tag="wyl")
                        nc.vector.tensor_tensor(out=wyl[:], in0=wtmp[(1, wyk)][:],
                                                in1=_ap(lwt, l, [[1, Q], [4, SP]]), op=AL.mult)
                        nc.vector.tensor_scalar(
                            _ap(IDX, l * 2 + rr, [[1, Q], [8, SP]]),
                            base[:], float(rr * Wl), None, AL.add)
                        for sh, wxk in ((0, 'A'), (1, 'B')):
                            nc.vector.tensor_tensor(
                                out=_ap(WS, l * 4 + rr * 2 + sh, [[1, Q], [16, SP]]),
                                in0=wyl[:], in1=wtmp[(0, wxk)][:], op=AL.mult)

                IDXi = sb2.tile([128, NSLOT], I16, tag="IDXi")
                nc.vector.tensor_copy(out=IDXi[:], in_=_ap(IDX, 0, [[1, 128], [1, NSLOT]]))
                nc.sync.dma_start(out=_ap(idx_d, (b * 3) * 128 * NSLOT,
                                          [[NSLOT, 128], [1, NSLOT]]), in_=IDXi[:])
                gwf = sbM.tile([128, NSLOT * 8], I16, tag="wrap")
                for rep in range(8):
                    nc.sync.dma_start(
                        out=gwf[rep * 16:(rep + 1) * 16, :],
                        in_=_ap(idx_d, (b * 3) * 128 * NSLOT,
                                [[NSLOT, 16], [1, NSLOT], [NSLOT * 16, 8]]))

                spat = sb2.tile([128, SP, CG], F32, tag="spat")
                for sc in range(32):
                    gat = sb3.tile([128, 8, 128], F32, tag="gat")
                    nc.gpsimd.dma_gather(
                        out_ap=gat[:],
                        in_ap=_ap(mslab_d, b * (NPIXP + 8) * CG, [[CG, NPIXP], [1, 128]]),
                        idxs_ap=_ap(gwf, sc * 64, [[1, 128], [1, 64]]),
                        num_idxs=1024, num_idxs_reg=1024,
                        elem_size=128, elem_step=CG)
                    tmp = sb3.tile([128, CG, 16], F32, tag="tmpg")
                    nc.vector.tensor_tensor(
                        out=_ap(tmp, 0, [[1, 128], [2, 8], [1, 2], [16, CG]]),
                        in0=_ap(gat, 0, [[1, 128], [128, 8], [64, 2], [1, CG]]),
                        in1=_ap(WS, sc * 16, [[1, 128], [2, 8], [1, 2], [0, CG]]),
                        op=AL.mult)
                    nc.vector.tensor_reduce(
                        out=spat[:, sc, :],
                        in_=tmp[:], axis=AX.X, op=AL.add)
                nc.sync.dma_start(
                    out=_ap(spatT_d, b * CG * Q * SP, [[SP, Q], [1, SP], [Q * SP, CG]]),
                    in_=spat[0:Q, :, :])
                if dbg:
                    nc.sync.dma_start(
                        out=_ap(d_spat, b * Q * SP * CG, [[SP * CG, Q], [CG, SP], [1, CG]]),
                        in_=spat[0:Q, :, :])

                # temporal coef scatter
                P4 = sb2.tile([128, NSLOT], F32, tag="P4")
                for qb in range(2):
                    nc.vector.tensor_scalar(
                        P4[:], _ap(IDX, 0, [[1, 128], [1, NSLOT]]),
                        4.0, float(4 + qb), AL.mult, AL.add)
                    pix_p = psA.tile([128, NSLOT], F32, tag="psa")
                    nc.tensor.matmul(pix_p[:], selS[qb][:], P4[0:Q, :], start=True, stop=True)
                    idxq = sb2.tile([128, NSLOT], F32, tag="idxq")
                    nc.vector.tensor_scalar(idxq[:], pix_p[:], shcS[:, 0:1], None, AL.add)
                    idxqi = sb2.tile([128, NSLOT], I16, tag="idxqi")
                    nc.vector.tensor_copy(out=idxqi[:], in_=idxq[:])
                    nc.sync.dma_start(out=_ap(idx_d, (b * 3 + 1 + qb) * 128 * NSLOT,
                                              [[NSLOT, 128], [1, NSLOT]]), in_=idxqi[:])
                    swf = sbM.tile([128, NSLOT * 8], I16, tag="wrap")
                    for rep in range(8):
                        nc.sync.dma_start(
                            out=swf[rep * 16:(rep + 1) * 16, :],
                            in_=_ap(idx_d, (b * 3 + 1 + qb) * 128 * NSLOT,
                                    [[NSLOT, 16], [1, NSLOT], [NSLOT * 16, 8]]))
                    wh = sb2.tile([128, NSLOT], F32, tag="wh")
                    for sh in range(2):
                        wp = psA.tile([128, NSLOT], F32, tag="psa")
                        nc.tensor.matmul(
                            wp[:], selS[qb][:],
                            _ap(WS, sh, [[1, Q], [2, NSLOT]]),
                            start=True, stop=True)
                        # WS already has the 1/8 fold; need /SP overall => *8/SP
                        nc.scalar.activation(out=wh[sh * 64:(sh + 1) * 64, :],
                                             in_=wp[sh * 64:(sh + 1) * 64, :],
                                             func=AF.Copy, scale=8.0 / SP)
                    for c0 in range(16):
                        oh = sb3.tile([128, 16, 64], F32, tag="oh")
                        nc.vector.tensor_tensor(
                            out=oh[:],
                            in0=_ap(ohbS, 0, [[1, 128], [0, 16], [1, 64]]),
                            in1=_ap(wh, c0 * 16, [[1, 128], [1, 16], [0, 64]]),
                            op=AL.mult)
                        nc.gpsimd.dma_scatter_add(
                            out_ap=_ap(co2_d, b * CO2ROWS * 64, [[64, CO2ROWS], [1, 64]]),
                            in_ap=oh[:],
                            idxs_ap=_ap(swf, c0 * 128, [[1, 128], [1, 128]]),
                            num_idxs=2048, num_idxs_reg=2048, elem_size=64)

                coefS = sbM.tile([128, NCH, 128], BF16, tag="med")
                for qb in range(2):
                    nc.gpsimd.dma_start(
                        out=_ap(coefS, qb * 64, [[1, 128], [128, NCH], [1, 64]]),
                        in_=_ap(co2_d, (b * CO2ROWS + 4 + qb) * 64,
                                [[256, 128], [256 * 128, NCH], [1, 64]]))
                    for hh in range(2):
                        nh = 22 if hh == 0 else NCH - 22
                        Bv = sb4.tile([128, 22, 64], BF16, tag="Bc")
                        nc.gpsimd.dma_start(
                            out=Bv[0:128, 0:nh, :],
                            in_=_ap(co2_d, (b * CO2ROWS + 2 + qb + hh * 22 * 512) * 64,
                                    [[256, 128], [256 * 128, nh], [1, 64]]))
                        nc.vector.tensor_tensor(
                            out=_ap(coefS, (hh * 22 * 128) + qb * 64,
                                    [[1, 128], [128, nh], [1, 64]]),
                            in0=_ap(coefS, (hh * 22 * 128) + qb * 64,
                                    [[1, 128], [128, nh], [1, 64]]),
                            in1=Bv[0:128, 0:nh, :], op=AL.add)
                ptm = psB.tile([Q, T * CG], F32, tag="psb")
                for g8 in range(6):
                    sg, n = slab_group(b, g8)
                    for kk in range(n):
                        k = g8 * 8 + kk
                        nc.tensor.matmul(ptm[:], _ap(coefS, k * 128, [[1, 128], [1, Q]]),
                                         sg[:, kk, :], start=(k == 0), stop=(k == NCH - 1))
                tmps = sb2.tile([Q, T * CG], F32, tag="tmps")
                nc.scalar.activation(out=tmps[:], in_=ptm[:], func=AF.Copy)
                nc.sync.dma_start(
                    out=_ap(tempT_d, b * CG * Q * TP, [[TP, Q], [1, TP], [Q * TP, CG]]),
                    in_=tmps[:])
                if dbg:
                    nc.sync.dma_start(out=_ap(d_temp, b * Q * T * CG,
                                              [[T * CG, Q], [1, T * CG]]), in_=tmps[:])

            # ============ mixing ============
            for ty in range(2):
                P_, OUT_, PSZ = (SP, OUT_S, PS) if ty == 0 else (TP, OUT_T, PT)
                xdram = spatT_d if ty == 0 else tempT_d
                wsrc = w_ps if ty == 0 else w_pt
                wodram = wod_s if ty == 0 else wod_t
                for b in range(2):
                    parS = sbS.tile([Q, PS], BF16, tag="big")
                    for nkc in range(PSZ // 512):
                        wc = sb3.tile([128, 2, 512], BF16, tag="wc")
                        nc.gpsimd.dma_start(out=wc[:], in_=_ap(wsrc, nkc * 512,
                                            [[PSZ, 128], [PSZ * 128, 2], [1, 512]]))
                        pp = psB.tile([Q, 512], F32, tag="psb")
                        for ch in range(2):
                            nc.tensor.matmul(pp[:], qTs[(ty, b, 'b')][:, ch, 0:Q],
                                             wc[:, ch, :],
                                             start=(ch == 0), stop=(ch == 1))
                        nc.scalar.activation(out=parS[:, nkc * 512:(nkc + 1) * 512],
                                             in_=pp[:], func=AF.Copy)
                    nc.sync.dma_start(
                        out=_ap(par_d, (ty * 2 + b) * Q * PS, [[PS, Q], [1, PSZ]]),
                        in_=parS[:, 0:PSZ])
                    # S part (host p-major) -> sT_d [P_, Q, OUT_]
                    nc.sync.dma_start(
                        out=_ap(sT_d, 0, [[OUT_, Q], [Q * OUT_, P_], [1, OUT_]]),
                        in_=_ap(parS, EFF * EFF, [[1, Q], [OUT_, P_], [1, OUT_]]))
                    Mall = sbM.tile([EFF, Q, EFF], BF16, tag="med")
                    nc.sync.dma_start(out=Mall[:], in_=_ap(par_d, (ty * 2 + b) * Q * PS,
                                      [[EFF, EFF], [PS, Q], [1, EFF]]))
                    SallT = sbS.tile([SP, Q, OUT_S], BF16, tag="big")
                    nc.sync.dma_start(out=SallT[0:P_, 0:Q, 0:OUT_],
                                      in_=_ap(sT_d, 0, [[Q * OUT_, P_], [1, Q * OUT_]]))
                    xT = sbM.tile([CG, Q, SP], BF16, tag="xT")
                    nc.gpsimd.dma_start(out=xT[0:CG, 0:Q, 0:P_],
                                        in_=_ap(xdram, b * Q * P_ * CG,
                                        [[Q * P_, CG], [1, Q * P_]]))
                    # M-mix (swapped): out1 [P_ s-part, EFF] per q
                    o1sb = sb2.tile([SP, Q, EFF], BF16, tag="o1sb")
                    for qg in range(0, Q, 8):
                        qn = min(8, Q - qg)
                        pg = psB.tile([SP, 8, EFF], F32, tag="psb")
                        for qi in range(qn):
                            q_ = qg + qi
                            nc.tensor.matmul(pg[0:P_, qi, :],
                                             _ap(xT, q_ * SP, [[1, CG], [1, P_]]),
                                             _ap(Mall, q_ * EFF, [[1, EFF], [1, EFF]]),
                                             start=True, stop=False)
                            nc.tensor.matmul(pg[0:P_, qi, :],
                                             _ap(xT, q_ * SP, [[1, CG], [1, P_]]),
                                             mbSb[ty][:], start=False, stop=True)
                        nc.scalar.activation(out=o1sb[0:P_, qg:qg + qn, :],
                                             in_=pg[0:P_, 0:qn, :], func=AF.Copy)
                    _pw_ln_relu(nc, sb2, sbM, psA, o1sb, P_, onesC, onesR, "a")
                    # S-mix (swapped): out [OUT_, EFF] per q, o on partitions
                    o2all = sbS.tile([OUT_S, Q, EFF], BF16, tag="big")
                    for qg in range(0, Q, 8):
                        qn = min(8, Q - qg)
                        pg = psB.tile([OUT_S, 8, EFF], F32, tag="psb")
                        for qi in range(qn):
                            q_ = qg + qi
                            nc.tensor.matmul(pg[0:OUT_, qi, :],
                                             _ap(SallT, q_ * OUT_S, [[1, P_], [1, OUT_]]),
                                             _ap(o1sb, q_ * EFF, [[1, P_], [1, EFF]]),
                                             start=True, stop=False)
                            nc.tensor.matmul(pg[0:OUT_, qi, :], sbtSb[ty][:],
                                             _ap(o1sb, q_ * EFF, [[1, P_], [1, EFF]]),
                                             start=False, stop=True)
                        nc.scalar.activation(out=o2all[0:OUT_, qg:qg + qn, :],
                                             in_=pg[0:OUT_, 0:qn, :], func=AF.Copy)
                    _pw_ln_relu(nc, sb2, sbM, psA, o2all, OUT_, onesC, onesR, "b")
                    # out-proj: contract o via 64 per-d matmuls, Wo host-permuted d-major
                    po = psB.tile([Q, D], F32, tag="psb")
                    for dg in range(8):
                        wo = sb3.tile([128, 8, D], BF16, tag="wo")
                        nc.gpsimd.dma_start(
                            out=wo[0:OUT_, :, :],
                            in_=_ap(wodram, dg * 8 * OUT_ * D,
                                    [[D, OUT_], [D * OUT_, 8], [1, D]]))
                        for j in range(8):
                            d_ = dg * 8 + j
                            nc.tensor.matmul(po[:],
                                             _ap(o2all, d_, [[1, OUT_], [EFF, Q]]),
                                             wo[0:OUT_, j, :],
                                             start=(d_ == 0), stop=(d_ == 63))
                    posb = sb2.tile([Q, D], F32, tag="posb")
                    nc.scalar.activation(out=posb[:], in_=po[:], func=AF.Copy)
                    nc.sync.dma_start(out=_ap(ar_in, (ty * 2 + b) * Q * D,
                                              [[D, Q], [1, D]]), in_=posb[:])

            nc.gpsimd.collective_compute(
                "AllReduce", AL.add,
                replica_groups=[[0, 1, 2, 3], [4, 5, 6, 7]],
                ins=[ar_in], outs=[ar_out])
            for ty in range(2):
                for b in range(2):
                    acc = sb2.tile([Q, D], F32, tag="acc")
                    nc.sync.dma_start(out=acc[:], in_=_ap(ar_out, (ty * 2 + b) * Q * D,
                                      [[D, Q], [1, D]]))
                    qt2 = sb2.tile([Q, D], F32, tag="qt2")
                    nc.sync.dma_start(out=qt2[:], in_=_ap((sqin if ty == 0 else tqin),
                                      b * Q * D, [[D, Q], [1, D]]))
                    nc.vector.tensor_tensor(out=acc[:], in0=acc[:], in1=qt2[:], op=AL.add)
                    nc.vector.tensor_tensor(out=acc[:], in0=acc[:], in1=bobcS[ty][0:Q, :], op=AL.add)
                    on = sb2.tile([Q, D], F32, tag="on")
                    al, be = _stats(nc, sb2, acc[:], D, Q, "n")
                    nc.vector.tensor_scalar(on[:], acc[:], al[:, 0:1], be[:, 0:1],
                                            AL.mult, AL.add)
                    nc.vector.tensor_tensor(out=on[:], in0=on[:], in1=gbcS[ty][0:Q, :], op=AL.mult)
                    nc.vector.tensor_tensor(out=on[:], in0=on[:], in1=bebcS[ty][0:Q, :], op=AL.add)
                    nc.sync.dma_start(out=_ap(out, (ty * 2 + b) * Q * D,
                                              [[D, Q], [1, D]]), in_=on[:])
    nc.compile()
    return nc


# ======================= host side =======================
_NC_CACHE = {}


def _get_nc(dbg=False):
    if dbg not in _NC_CACHE:
        _NC_CACHE[dbg] = build_nc(dbg)
    return _NC_CACHE[dbg]


def make_consts():
    c = {}
    c['ident'] = np.eye(128, dtype=np.float32)
    ohb = np.zeros((128, 64), np.float32)
    ohb[np.arange(128), np.arange(128) % 64] = 1.0
    c['ohbase'] = ohb
    sc = np.zeros((128, 1), np.float32); sc[64:] = 2.0
    c['shcol'] = sc
    c['onesin'] = np.ones((128, 1), np.float32)
    for qb in range(2):
        sel = np.zeros((Q, 128), np.float32)
        for q in range(qb * 64, min((qb + 1) * 64, Q)):
            sel[q, q - qb * 64] = 1.0
            sel[q, q - qb * 64 + 64] = 1.0
        c[f'sel{qb}'] = sel
    return c


def _bf(a):
    return np.ascontiguousarray(np.asarray(a).astype(ml_dtypes.bfloat16))


def make_in_map(inputs, g, bp, consts):
    inp = {k: np.ascontiguousarray(np.asarray(v), dtype=np.float32)
           for k, v in inputs.items()}
    bsel = [2 * bp, 2 * bp + 1]
    slabs = np.zeros((2, NPIXP, T * CG), ml_dtypes.bfloat16)
    for i, b in enumerate(bsel):
        for l in range(4):
            a = inp['feat%d' % l][b, g * CG:(g + 1) * CG]
            a = a.transpose(2, 3, 1, 0).reshape(HWL[l] * HWL[l], T * CG)
            slabs[i, LOFF[l]:LOFF[l] + a.shape[0]] = a.astype(ml_dtypes.bfloat16)
    perm = np.arange(96).reshape(32, 3).T.reshape(-1)
    m = dict(consts)
    m['slabs'] = slabs
    m['sq'] = inp['spatial_queries'][bsel].copy()
    m['tq'] = inp['temporal_queries'][bsel].copy()
    m['boxes'] = inp['proposal_boxes'][bsel].copy()
    m['w_off'] = np.ascontiguousarray(inp['W_off'][:, g * 96:(g + 1) * 96][:, perm])
    m['boffbc'] = np.broadcast_to(inp['b_off'][g * 96:(g + 1) * 96][perm],
                                  (128, 96)).copy()
    wps = inp['W_ps'][:, g * PS:(g + 1) * PS].copy()
    wps[:, EFF * EFF:] = wps[:, EFF * EFF:].reshape(D, OUT_S, SP).transpose(
        0, 2, 1).reshape(D, SP * OUT_S)
    m['w_ps'] = np.ascontiguousarray(wps)
    wpt = inp['W_pt'][:, g * PT:(g + 1) * PT].copy()
    wpt[:, EFF * EFF:] = wpt[:, EFF * EFF:].reshape(D, OUT_T, TP).transpose(
        0, 2, 1).reshape(D, TP * OUT_T)
    m['w_pt'] = np.ascontiguousarray(wpt)
    wos = inp['Wo_s'][g * EFF * OUT_S:(g + 1) * EFF * OUT_S]
    m['wod_s'] = np.ascontiguousarray(
        wos.reshape(OUT_S, EFF, D).transpose(1, 0, 2).reshape(EFF * OUT_S, D))
    wot = inp['Wo_t'][g * EFF * OUT_T:(g + 1) * EFF * OUT_T]
    m['wod_t'] = np.ascontiguousarray(
        wot.reshape(OUT_T, EFF, D).transpose(1, 0, 2).reshape(EFF * OUT_T, D))
    bps = inp['b_ps'][g * PS:(g + 1) * PS]
    bpt = inp['b_pt'][g * PT:(g + 1) * PT]
    m['mb_s'] = _bf(bps[:EFF * EFF].reshape(EFF, EFF))
    m['sbt_s'] = _bf(bps[EFF * EFF:].reshape(OUT_S, SP).T)
    m['mb_t'] = _bf(bpt[:EFF * EFF].reshape(EFF, EFF))
    m['sbt_t'] = _bf(bpt[EFF * EFF:].reshape(OUT_T, TP).T)
    for nm, key in (('gsbc', 'g_s'), ('besbc', 'be_s'), ('gtbc', 'g_t'),
                    ('betbc', 'be_t'), ('bosbc', 'bo_s'), ('botbc', 'bo_t')):
        m[nm] = np.broadcast_to(inp[key], (128, D)).copy()
    return m


def run_cores(inputs, dbg=False):
    nc = _get_nc(dbg)
    consts = make_consts()
    in_maps = [make_in_map(inputs, c % 4, c // 4, consts) for c in range(N_CORES)]
    return bass_utils.run_bass_kernel_spmd(nc, in_maps, core_ids=list(range(N_CORES)))


def kernel(**inputs):
    res = run_cores(inputs, dbg=False)
    sq = np.zeros((4, Q, D), np.float32)
    tq = np.zeros((4, Q, D), np.float32)
    for bp, core in ((0, 0), (1, 4)):
        o = res.results[core]["out"]
        sq[2 * bp:2 * bp + 2] = o[0]
        tq[2 * bp:2 * bp + 2] = o[1]
    return sq, tq
